# revision 4
# baseline (speedup 1.0000x reference)
"""BiGCN layer kernel for 8 Trainium2 NeuronCores.

Strategy (column-parallel SpMM + ReduceScatter):
  - Each core c owns the contraction slice n in [c*512, (c+1)*512) of all six
    adjacency matrices (3 bw + 3 fw), pre-transposed on host to [n_loc, m] so
    that the contraction dim lands on SBUF partitions with no on-chip
    transposes.
  - sup[r] = inps @ W[r] is computed locally per core for its n-slice only
    (lhsT = inps[block].T, uploaded once and reused for the residual add).
  - feats^T partial = sum_r sup_slice[r].T-contract adjT[r] accumulates all
    relations directly in PSUM; a ReduceScatter over the 8 cores sums the
    partials and hands core c its own m-block.
  - bias+relu fuse into one scalar-engine activation (bias is per-partition
    since feats is produced transposed: [h, m]); final linear contracts h with
    W1 tiles as stationary operands; residual adds inps^T; each core writes
    its [512, 512] transposed output block, assembled on host.
"""

import numpy as np

N, H, R = 4096, 512, 3
K = H // 2            # 256
NC = 8                # cores
NB = N // NC          # 512 rows (m / n_loc) per core
MC = 1024             # m-chunk width streamed per PSUM accumulation group
F32 = None            # set lazily to mybir.dt.float32

_BUILT = {}


def _build_nc():
    """Build (and cache) the Bass program. Identical program on all 8 cores."""
    if "nc" in _BUILT:
        return _BUILT["nc"]

    import concourse.bass as bass
    import concourse.mybir as mybir
    from concourse import bacc, tile

    f32 = mybir.dt.float32
    nc = bacc.Bacc(None, num_devices=NC)

    inpsT = nc.dram_tensor("inpsT", [H, NB], f32, kind="ExternalInput")
    adjT = nc.dram_tensor("adjT", [2 * R, NB, N], f32, kind="ExternalInput")
    wst = nc.dram_tensor("wst", [2 * R, H, K], f32, kind="ExternalInput")
    bstack = nc.dram_tensor("bstack", [4, 128, R], f32, kind="ExternalInput")
    w1 = nc.dram_tensor("w1", [H, H], f32, kind="ExternalInput")
    b1s = nc.dram_tensor("b1s", [4, 128, 1], f32, kind="ExternalInput")
    outT = nc.dram_tensor("outT", [H, NB], f32, kind="ExternalOutput")

    HT = H // 128     # 4 h-tiles
    NT = NB // 128    # 4 n_loc tiles
    JT = H // 128     # 4 output j tiles
    NMC = N // MC     # 4 m chunks
    Relu = mybir.ActivationFunctionType.Relu
    Identity = mybir.ActivationFunctionType.Identity

    with tile.TileContext(nc) as tc:
        with (
            tc.tile_pool(name="const", bufs=1) as const,
            tc.tile_pool(name="adjp", bufs=3) as adjp,
            tc.tile_pool(name="evacp", bufs=3) as evacp,
            tc.tile_pool(name="psum", bufs=4, space=bass.MemorySpace.PSUM) as psump,
            tc.tile_pool(name="dram", bufs=1, space="DRAM") as dramp,
        ):
            # ---------------- constants into SBUF ----------------
            inpsT_sb = const.tile([128, HT, NB], f32)       # [p_h, ht, n_loc/m_loc]
            nc.sync.dma_start(inpsT_sb[:], inpsT[:, :].rearrange("(t p) n -> p t n", p=128))
            wst_sb = const.tile([128, 2 * R, HT, K], f32)   # [p_h, r, ht, k]
            nc.sync.dma_start(wst_sb[:], wst[:, :, :].rearrange("r (t p) k -> p r t k", p=128))
            w1_sb = const.tile([128, HT, H], f32)           # [p_h, ht, j]
            nc.sync.dma_start(w1_sb[:], w1[:, :].rearrange("(t p) j -> p t j", p=128))
            bst_sb = const.tile([128, JT, R], f32)
            nc.sync.dma_start(bst_sb[:], bstack[:, :, :].rearrange("t p r -> p t r"))
            b1_sb = const.tile([128, JT], f32)
            nc.sync.dma_start(b1_sb[:], b1s[:, :, :].rearrange("t p o -> p (t o)"))

            # summed (over relations) concat bias, per (p, jt)
            bias_sb = const.tile([128, JT], f32)
            for jt in range(JT):
                nc.vector.tensor_add(
                    bias_sb[:, jt : jt + 1], bst_sb[:, jt, 0:1], bst_sb[:, jt, 1:2]
                )
                nc.vector.tensor_add(
                    bias_sb[:, jt : jt + 1], bias_sb[:, jt : jt + 1], bst_sb[:, jt, 2:3]
                )

            # ---------------- local supports: sup[r][n_loc, k] ----------------
            sup_sb = const.tile([128, 2 * R, NT, K], f32)   # [p_n, r, nt, k]
            for r in range(2 * R):
                for nt in range(NT):
                    ps = psump.tile([128, K], f32, tag="pb")
                    for ht in range(HT):
                        nc.tensor.matmul(
                            ps[:],
                            inpsT_sb[:, ht, nt * 128 : (nt + 1) * 128],
                            wst_sb[:, r, ht, :],
                            start=(ht == 0),
                            stop=(ht == HT - 1),
                        )
                    nc.vector.tensor_copy(sup_sb[:, r, nt, :], ps[:])

            # ---------------- adjacency stream + RS staging ----------------
            stag = dramp.tile([2, NC, K, NB], f32)          # [dir, dest, k, m_loc]
            rs_out = []
            for dirn in range(2):                           # 0 = bw (h 0:256), 1 = fw
                for mc in range(NMC):
                    ps0 = psump.tile([128, MC], f32, tag="pb")   # k 0:128
                    ps1 = psump.tile([128, MC], f32, tag="pb")   # k 128:256
                    for ri in range(R):
                        r = dirn * R + ri
                        for nt in range(NT):
                            at = adjp.tile([128, MC], f32, tag="adj")
                            nc.sync.dma_start(
                                at[:],
                                adjT[r, nt * 128 : (nt + 1) * 128, mc * MC : (mc + 1) * MC],
                            )
                            first = ri == 0 and nt == 0
                            last = ri == R - 1 and nt == NT - 1
                            for kk, ps in ((0, ps0), (1, ps1)):
                                for mh in range(MC // 512):
                                    nc.tensor.matmul(
                                        ps[:, mh * 512 : (mh + 1) * 512],
                                        sup_sb[:, r, nt, kk * 128 : (kk + 1) * 128],
                                        at[:, mh * 512 : (mh + 1) * 512],
                                        start=first,
                                        stop=last,
                                    )
                    for kk, ps in ((0, ps0), (1, ps1)):
                        ev = evacp.tile([128, MC], f32, tag="ev")
                        nc.vector.tensor_copy(ev[:], ps[:])
                        for d2 in range(MC // NB):
                            dest = (mc * MC) // NB + d2
                            nc.sync.dma_start(
                                stag[dirn, dest, kk * 128 : (kk + 1) * 128, :],
                                ev[:, d2 * NB : (d2 + 1) * NB],
                            )
                ro = dramp.tile([1, K, NB], f32, name=f"rs_out{dirn}")
                rs_out.append(ro)
                nc.gpsimd.collective_compute(
                    "ReduceScatter",
                    mybir.AluOpType.add,
                    replica_groups=[list(range(NC))],
                    ins=[stag[dirn].opt()],
                    outs=[ro[:].opt()],
                )

            # ---------------- bias + relu + final linear + residual ----------------
            frelu_sb = const.tile([128, HT, NB], f32)       # [p_h, ht, m_loc]
            for jt in range(HT):
                dirn, row = jt // 2, (jt % 2) * 128
                ft = evacp.tile([128, NB], f32, tag="ftmp")
                nc.sync.dma_start(ft[:], rs_out[dirn][0, row : row + 128, :])
                nc.scalar.activation(
                    frelu_sb[:, jt, :], ft[:], Relu, bias=bias_sb[:, jt : jt + 1]
                )
            for jt in range(JT):
                pso = psump.tile([128, NB], f32, tag="pb")
                for ht in range(HT):
                    nc.tensor.matmul(
                        pso[:],
                        w1_sb[:, ht, jt * 128 : (jt + 1) * 128],
                        frelu_sb[:, ht, :],
                        start=(ht == 0),
                        stop=(ht == HT - 1),
                    )
                ot = evacp.tile([128, NB], f32, tag="ev")
                nc.scalar.activation(
                    ot[:], pso[:], Identity, bias=b1_sb[:, jt : jt + 1]
                )
                nc.vector.tensor_add(ot[:], ot[:], inpsT_sb[:, jt, :])
                nc.sync.dma_start(outT[jt * 128 : (jt + 1) * 128, :], ot[:])

    nc.compile()
    nc.finalize()
    _BUILT["nc"] = nc
    return nc


def _make_in_maps(inps, fw_adjs, bw_adjs, W_fw, b_fw, W_bw, b_bw, W1, b1):
    f = np.float32
    inps = np.asarray(inps, f)
    W1 = np.ascontiguousarray(np.asarray(W1, f))
    wst = np.ascontiguousarray(
        np.concatenate([np.asarray(W_bw, f), np.asarray(W_fw, f)], axis=0)
    )
    b_cat = np.concatenate([np.asarray(b_bw, f), np.asarray(b_fw, f)], axis=1)  # [R, H]
    bstack = np.ascontiguousarray(b_cat.T.reshape(4, 128, R))
    b1s = np.ascontiguousarray(np.asarray(b1, f).reshape(4, 128, 1))
    fw_adjs = np.asarray(fw_adjs, f)
    bw_adjs = np.asarray(bw_adjs, f)

    in_maps = []
    for c in range(NC):
        sl = slice(c * NB, (c + 1) * NB)
        adjT_c = np.empty((2 * R, NB, N), f)
        for r in range(R):
            adjT_c[r] = bw_adjs[r][:, sl].T
            adjT_c[R + r] = fw_adjs[r][:, sl].T
        in_maps.append(
            {
                "inpsT": np.ascontiguousarray(inps[sl].T),
                "adjT": adjT_c,
                "wst": wst,
                "bstack": bstack,
                "w1": W1,
                "b1s": b1s,
            }
        )
    return in_maps


def run(trace=False, **inputs):
    """Run the SPMD kernel; returns (full_output, BassKernelResults)."""
    from concourse.bass_utils import run_bass_kernel_spmd

    nc = _build_nc()
    in_maps = _make_in_maps(**inputs)
    res = run_bass_kernel_spmd(nc, in_maps, core_ids=list(range(NC)), trace=trace)
    out = np.empty((N, H), np.float32)
    for c in range(NC):
        out[c * NB : (c + 1) * NB] = res.results[c]["outT"].T
    return out, res


def kernel(**inputs):
    out, _ = run(trace=False, **inputs)
    return out


# revision 16
# speedup vs baseline: 1.6607x; 1.6607x over previous
"""BiGCN layer kernel for 8 Trainium2 NeuronCores.

Strategy (column-parallel SpMM + ReduceScatter):
  - Each core c owns the contraction slice n in [c*512, (c+1)*512) of all six
    adjacency matrices (3 bw + 3 fw), pre-transposed on host to [n_loc, m] so
    that the contraction dim lands on SBUF partitions with no on-chip
    transposes.
  - sup[r] = inps @ W[r] is computed locally per core for its n-slice only
    (lhsT = inps[block].T, uploaded once and reused for the residual add).
  - feats^T partial = sum_r sup_slice[r].T-contract adjT[r] accumulates all
    relations directly in PSUM; a ReduceScatter over the 8 cores sums the
    partials and hands core c its own m-block.
  - bias+relu fuse into one scalar-engine activation (bias is per-partition
    since feats is produced transposed: [h, m]); final linear contracts h with
    W1 tiles as stationary operands; residual adds inps^T; each core writes
    its [512, 512] transposed output block, assembled on host.
"""

import numpy as np

N, H, R = 4096, 512, 3
K = H // 2            # 256
NC = 8                # cores
NB = N // NC          # 512 rows (m / n_loc) per core
MC = 1024             # m-chunk width streamed per PSUM accumulation group
F32 = None            # set lazily to mybir.dt.float32

_BUILT = {}


def _build_nc():
    """Build (and cache) the Bass program. Identical program on all 8 cores."""
    if "nc" in _BUILT:
        return _BUILT["nc"]

    import concourse.bass as bass
    import concourse.mybir as mybir
    from concourse import bacc, tile

    f32 = mybir.dt.float32
    f32r = mybir.dt.float32r
    nc = bacc.Bacc(None, num_devices=NC)

    inpsT = nc.dram_tensor("inpsT", [H, NB], f32r, kind="ExternalInput")
    adjT = nc.dram_tensor("adjT", [2 * R, NB, N], f32r, kind="ExternalInput")
    wst = nc.dram_tensor("wst", [2 * R, H, K], f32r, kind="ExternalInput")
    bstack = nc.dram_tensor("bstack", [4, 128, R], f32, kind="ExternalInput")
    w1 = nc.dram_tensor("w1", [H, H], f32, kind="ExternalInput")
    b1s = nc.dram_tensor("b1s", [4, 128, 1], f32, kind="ExternalInput")
    outT = nc.dram_tensor("outT", [H, NB], f32, kind="ExternalOutput")

    HT = H // 128     # 4 h-tiles
    NT = NB // 128    # 4 n_loc tiles
    JT = H // 128     # 4 output j tiles
    NMC = N // MC     # 4 m chunks
    Relu = mybir.ActivationFunctionType.Relu
    Identity = mybir.ActivationFunctionType.Identity

    with tile.TileContext(nc) as tc:
        with (
            tc.tile_pool(name="const", bufs=1) as const,
            tc.tile_pool(name="adjp", bufs=3) as adjp,
            tc.tile_pool(name="evacp", bufs=3) as evacp,
            tc.tile_pool(name="psum", bufs=4, space=bass.MemorySpace.PSUM) as psump,
            tc.tile_pool(name="dram", bufs=1, space="DRAM") as dramp,
        ):
            # ---------------- constants into SBUF ----------------
            inpsT_sb = const.tile([128, HT, NB], f32r)      # [p_h, ht, n_loc/m_loc]
            nc.sync.dma_start(inpsT_sb[:], inpsT[:, :].rearrange("(t p) n -> p t n", p=128))
            wst_sb = const.tile([128, 2 * R, HT, K], f32r)  # [p_h, r, ht, k]
            nc.sync.dma_start(wst_sb[:], wst[:, :, :].rearrange("r (t p) k -> p r t k", p=128))
            w1_sb = const.tile([128, HT, H], f32)           # [p_h, ht, j]
            nc.sync.dma_start(w1_sb[:], w1[:, :].rearrange("(t p) j -> p t j", p=128))
            bst_sb = const.tile([128, JT, R], f32)
            nc.sync.dma_start(bst_sb[:], bstack[:, :, :].rearrange("t p r -> p t r"))
            b1_sb = const.tile([128, JT], f32)
            nc.sync.dma_start(b1_sb[:], b1s[:, :, :].rearrange("t p o -> p (t o)"))

            # summed (over relations) concat bias, per (p, jt)
            bias_sb = const.tile([128, JT], f32)
            for jt in range(JT):
                nc.vector.tensor_add(
                    bias_sb[:, jt : jt + 1], bst_sb[:, jt, 0:1], bst_sb[:, jt, 1:2]
                )
                nc.vector.tensor_add(
                    bias_sb[:, jt : jt + 1], bias_sb[:, jt : jt + 1], bst_sb[:, jt, 2:3]
                )

            # ---------------- local supports: sup[r][n_loc, k] ----------------
            sup_sb = const.tile([128, 2 * R, NT, K], f32r)  # [p_n, r, nt, k]
            for r in range(2 * R):
                for nt in range(NT):
                    ps = psump.tile([128, K], f32, tag="pb")
                    for ht in range(HT):
                        nc.tensor.matmul(
                            ps[:],
                            inpsT_sb[:, ht, nt * 128 : (nt + 1) * 128],
                            wst_sb[:, r, ht, :],
                            start=(ht == 0),
                            stop=(ht == HT - 1),
                        )
                    nc.vector.tensor_copy(sup_sb[:, r, nt, :], ps[:])

            # ---------------- adjacency stream + RS staging ----------------
            stag = dramp.tile([2, NC, K, NB], f32)          # [dir, dest, k, m_loc]
            rs_out = []
            for dirn in range(2):                           # 0 = bw (h 0:256), 1 = fw
                for mc in range(NMC):
                    ps0 = psump.tile([128, MC], f32, tag="pb")   # k 0:128
                    ps1 = psump.tile([128, MC], f32, tag="pb")   # k 128:256
                    for ri in range(R):
                        r = dirn * R + ri
                        at = adjp.tile([128, NT, MC], f32r, tag="adj")
                        nc.sync.dma_start(
                            at[:],
                            adjT[r, :, mc * MC : (mc + 1) * MC].rearrange(
                                "(t p) m -> p t m", p=128
                            ),
                        )
                        for nt in range(NT):
                            first = ri == 0 and nt == 0
                            last = ri == R - 1 and nt == NT - 1
                            for kk, ps in ((0, ps0), (1, ps1)):
                                lhsT = sup_sb[:, r, nt, kk * 128 : (kk + 1) * 128]
                                for mh in range(MC // 512):
                                    nc.tensor.matmul(
                                        ps[:, mh * 512 : (mh + 1) * 512],
                                        lhsT,
                                        at[:, nt, mh * 512 : (mh + 1) * 512],
                                        start=first,
                                        stop=last,
                                    )
                    for kk, ps in ((0, ps0), (1, ps1)):
                        ev = evacp.tile([128, MC], f32, tag="ev")
                        nc.vector.tensor_copy(ev[:], ps[:])
                        for d2 in range(MC // NB):
                            dest = (mc * MC) // NB + d2
                            nc.sync.dma_start(
                                stag[dirn, dest, kk * 128 : (kk + 1) * 128, :],
                                ev[:, d2 * NB : (d2 + 1) * NB],
                            )
                ro = dramp.tile([1, K, NB], f32, name=f"rs_out{dirn}")
                rs_out.append(ro)
                nc.gpsimd.collective_compute(
                    "ReduceScatter",
                    mybir.AluOpType.add,
                    replica_groups=[list(range(NC))],
                    ins=[stag[dirn].opt()],
                    outs=[ro[:].opt()],
                )

            # ---------------- bias + relu + final linear + residual ----------------
            frelu_sb = const.tile([128, HT, NB], f32)       # [p_h, ht, m_loc]
            for jt in range(HT):
                dirn, row = jt // 2, (jt % 2) * 128
                ft = evacp.tile([128, NB], f32, tag="ftmp")
                nc.sync.dma_start(ft[:], rs_out[dirn][0, row : row + 128, :])
                nc.scalar.activation(
                    frelu_sb[:, jt, :], ft[:], Relu, bias=bias_sb[:, jt : jt + 1]
                )
            for jt in range(JT):
                pso = psump.tile([128, NB], f32, tag="pb")
                for ht in range(HT):
                    nc.tensor.matmul(
                        pso[:],
                        w1_sb[:, ht, jt * 128 : (jt + 1) * 128],
                        frelu_sb[:, ht, :],
                        start=(ht == 0),
                        stop=(ht == HT - 1),
                    )
                ot = evacp.tile([128, NB], f32, tag="ev")
                nc.scalar.activation(
                    ot[:], pso[:], Identity, bias=b1_sb[:, jt : jt + 1]
                )
                nc.vector.tensor_add(ot[:], ot[:], inpsT_sb[:, jt, :].bitcast(f32))
                nc.sync.dma_start(outT[jt * 128 : (jt + 1) * 128, :], ot[:])

    nc.compile()
    nc.finalize()
    _BUILT["nc"] = nc
    return nc


def _round_fp32r(a):
    """Round fp32 to the fp32r (TF32-like, 1s+8e+11m in top 20 bits) format
    with round-to-nearest-even, as the PE's fp32r datapath expects."""
    b = np.ascontiguousarray(a, np.float32).view(np.uint32).astype(np.uint64)
    lsb = (b >> 12) & 1
    r = ((b + 0x7FF + lsb) & 0xFFFFF000).astype(np.uint32)
    return r.view(np.float32)


def _make_in_maps(inps, fw_adjs, bw_adjs, W_fw, b_fw, W_bw, b_bw, W1, b1):
    f = np.float32
    inps = np.asarray(inps, f)
    W1 = np.ascontiguousarray(np.asarray(W1, f))
    wst = _round_fp32r(
        np.concatenate([np.asarray(W_bw, f), np.asarray(W_fw, f)], axis=0)
    )
    b_cat = np.concatenate([np.asarray(b_bw, f), np.asarray(b_fw, f)], axis=1)  # [R, H]
    bstack = np.ascontiguousarray(b_cat.T.reshape(4, 128, R))
    b1s = np.ascontiguousarray(np.asarray(b1, f).reshape(4, 128, 1))
    fw_adjs = np.asarray(fw_adjs, f)
    bw_adjs = np.asarray(bw_adjs, f)

    in_maps = []
    for c in range(NC):
        sl = slice(c * NB, (c + 1) * NB)
        adjT_c = np.empty((2 * R, NB, N), f)
        for r in range(R):
            adjT_c[r] = bw_adjs[r][:, sl].T
            adjT_c[R + r] = fw_adjs[r][:, sl].T
        in_maps.append(
            {
                "inpsT": _round_fp32r(inps[sl].T),
                "adjT": _round_fp32r(adjT_c),
                "wst": wst,
                "bstack": bstack,
                "w1": W1,
                "b1s": b1s,
            }
        )
    return in_maps


def run(trace=False, **inputs):
    """Run the SPMD kernel; returns (full_output, BassKernelResults)."""
    from concourse.bass_utils import run_bass_kernel_spmd

    nc = _build_nc()
    in_maps = _make_in_maps(**inputs)
    res = run_bass_kernel_spmd(nc, in_maps, core_ids=list(range(NC)), trace=trace)
    out = np.empty((N, H), np.float32)
    for c in range(NC):
        out[c * NB : (c + 1) * NB] = res.results[c]["outT"].T
    return out, res


def kernel(**inputs):
    out, _ = run(trace=False, **inputs)
    return out


# revision 23
# speedup vs baseline: 1.7681x; 1.0646x over previous
"""BiGCN layer kernel for 8 Trainium2 NeuronCores.

Strategy (column-parallel SpMM + ReduceScatter):
  - Each core c owns the contraction slice n in [c*512, (c+1)*512) of all six
    adjacency matrices (3 bw + 3 fw), pre-transposed on host to [n_loc, m] so
    that the contraction dim lands on SBUF partitions with no on-chip
    transposes.
  - sup[r] = inps @ W[r] is computed locally per core for its n-slice only
    (lhsT = inps[block].T, uploaded once and reused for the residual add).
  - feats^T partial = sum_r sup_slice[r].T-contract adjT[r] accumulates all
    relations directly in PSUM; a ReduceScatter over the 8 cores sums the
    partials and hands core c its own m-block.
  - bias+relu fuse into one scalar-engine activation (bias is per-partition
    since feats is produced transposed: [h, m]); final linear contracts h with
    W1 tiles as stationary operands; residual adds inps^T; each core writes
    its [512, 512] transposed output block, assembled on host.
"""

import numpy as np

N, H, R = 4096, 512, 3
K = H // 2            # 256
NC = 8                # cores
NB = N // NC          # 512 rows (m / n_loc) per core
MC = 1024             # m-chunk width streamed per PSUM accumulation group
F32 = None            # set lazily to mybir.dt.float32

_BUILT = {}


def _build_nc():
    """Build (and cache) the Bass program. Identical program on all 8 cores."""
    if "nc" in _BUILT:
        return _BUILT["nc"]

    import concourse.bass as bass
    import concourse.mybir as mybir
    from concourse import bacc, tile

    f32 = mybir.dt.float32
    f32r = mybir.dt.float32r
    nc = bacc.Bacc(None, num_devices=NC)

    inpsT = nc.dram_tensor("inpsT", [H, NB], f32r, kind="ExternalInput")
    adjT = nc.dram_tensor("adjT", [2 * R, NB, N], f32r, kind="ExternalInput")
    wst = nc.dram_tensor("wst", [2 * R, H, K], f32r, kind="ExternalInput")
    bstack = nc.dram_tensor("bstack", [4, 128, R], f32, kind="ExternalInput")
    w1 = nc.dram_tensor("w1", [H, H], f32r, kind="ExternalInput")
    b1s = nc.dram_tensor("b1s", [4, 128, 1], f32, kind="ExternalInput")
    outT = nc.dram_tensor("outT", [H, NB], f32, kind="ExternalOutput")

    HT = H // 128     # 4 h-tiles
    NT = NB // 128    # 4 n_loc tiles
    JT = H // 128     # 4 output j tiles
    NMC = N // MC     # 4 m chunks
    Relu = mybir.ActivationFunctionType.Relu
    Identity = mybir.ActivationFunctionType.Identity

    with tile.TileContext(nc) as tc:
        with (
            tc.tile_pool(name="const", bufs=1) as const,
            tc.tile_pool(name="adjp", bufs=4) as adjp,
            tc.tile_pool(name="evacp", bufs=3) as evacp,
            tc.tile_pool(name="psum", bufs=4, space=bass.MemorySpace.PSUM) as psump,
            tc.tile_pool(name="dram", bufs=1, space="DRAM") as dramp,
        ):
            # ---------------- constants into SBUF ----------------
            inpsT_sb = const.tile([128, HT, NB], f32r)      # [p_h, ht, n_loc/m_loc]
            nc.sync.dma_start(inpsT_sb[:], inpsT[:, :].rearrange("(t p) n -> p t n", p=128))
            wst_sb = const.tile([128, 2 * R, HT, K], f32r)  # [p_h, r, ht, k]
            nc.sync.dma_start(wst_sb[:], wst[:, :, :].rearrange("r (t p) k -> p r t k", p=128))
            w1_sb = const.tile([128, HT, H], f32r)          # [p_h, ht, j]
            nc.sync.dma_start(w1_sb[:], w1[:, :].rearrange("(t p) j -> p t j", p=128))
            bst_sb = const.tile([128, JT, R], f32)
            nc.sync.dma_start(bst_sb[:], bstack[:, :, :].rearrange("t p r -> p t r"))
            b1_sb = const.tile([128, JT], f32)
            nc.sync.dma_start(b1_sb[:], b1s[:, :, :].rearrange("t p o -> p (t o)"))

            # summed (over relations) concat bias, per (p, jt)
            bias_sb = const.tile([128, JT], f32)
            for jt in range(JT):
                nc.vector.tensor_add(
                    bias_sb[:, jt : jt + 1], bst_sb[:, jt, 0:1], bst_sb[:, jt, 1:2]
                )
                nc.vector.tensor_add(
                    bias_sb[:, jt : jt + 1], bias_sb[:, jt : jt + 1], bst_sb[:, jt, 2:3]
                )

            # ---------------- local supports: sup[r][n_loc, k] ----------------
            sup_sb = const.tile([128, 2 * R, NT, K], f32r)  # [p_n, r, nt, k]
            for r in range(2 * R):
                for nt in range(NT):
                    ps = psump.tile([128, K], f32, tag="pb")
                    for ht in range(HT):
                        nc.tensor.matmul(
                            ps[:],
                            inpsT_sb[:, ht, nt * 128 : (nt + 1) * 128],
                            wst_sb[:, r, ht, :],
                            start=(ht == 0),
                            stop=(ht == HT - 1),
                        )
                    nc.vector.tensor_copy(sup_sb[:, r, nt, :], ps[:])

            # ---------------- adjacency stream + RS staging ----------------
            # Two separate staging tensors: one shared tile would make the fw
            # stream's staging writes serialize behind RS_bw's read.
            stag0 = dramp.tile([NC, K, NB], f32, name="stag0", tag="stag0")
            stag1 = dramp.tile([NC, K, NB], f32, name="stag1", tag="stag1")
            stags = [stag0, stag1]
            rs_out = []
            for dirn in range(2):                           # 0 = bw (h 0:256), 1 = fw
                stag = stags[dirn]
                for mc in range(NMC):
                    ps0 = psump.tile([128, MC], f32, tag="pb")   # k 0:128
                    ps1 = psump.tile([128, MC], f32, tag="pb")   # k 128:256
                    for ri in range(R):
                        r = dirn * R + ri
                        at = adjp.tile([128, NT, MC], f32r, tag="adj")
                        nc.sync.dma_start(
                            at[:],
                            adjT[r, :, mc * MC : (mc + 1) * MC].rearrange(
                                "(t p) m -> p t m", p=128
                            ),
                        )
                        for nt in range(NT):
                            first = ri == 0 and nt == 0
                            last = ri == R - 1 and nt == NT - 1
                            for kk, ps in ((0, ps0), (1, ps1)):
                                lhsT = sup_sb[:, r, nt, kk * 128 : (kk + 1) * 128]
                                for mh in range(MC // 512):
                                    nc.tensor.matmul(
                                        ps[:, mh * 512 : (mh + 1) * 512],
                                        lhsT,
                                        at[:, nt, mh * 512 : (mh + 1) * 512],
                                        start=first,
                                        stop=last,
                                    )
                    for kk, ps in ((0, ps0), (1, ps1)):
                        ev = evacp.tile([128, MC], f32, tag="ev")
                        nc.vector.tensor_copy(ev[:], ps[:])
                        for d2 in range(MC // NB):
                            dest = (mc * MC) // NB + d2
                            nc.sync.dma_start(
                                stag[dest, kk * 128 : (kk + 1) * 128, :],
                                ev[:, d2 * NB : (d2 + 1) * NB],
                            )
                ro = dramp.tile(
                    [1, K, NB], f32, name=f"rs_out{dirn}", tag=f"rs_out{dirn}"
                )
                rs_out.append(ro)
                nc.gpsimd.collective_compute(
                    "ReduceScatter",
                    mybir.AluOpType.add,
                    replica_groups=[list(range(NC))],
                    ins=[stag[:].opt()],
                    outs=[ro[:].opt()],
                )

            # ---------------- bias + relu + final linear + residual ----------------
            frelu_sb = const.tile([128, HT, NB], f32r)      # [p_h, ht, m_loc]
            for jt in range(HT):
                dirn, row = jt // 2, (jt % 2) * 128
                ft = evacp.tile([128, NB], f32, tag="ftmp")
                nc.sync.dma_start(ft[:], rs_out[dirn][0, row : row + 128, :])
                nc.scalar.activation(
                    frelu_sb[:, jt, :], ft[:], Relu, bias=bias_sb[:, jt : jt + 1]
                )
            for jt in range(JT):
                pso = psump.tile([128, NB], f32, tag="pb")
                for ht in range(HT):
                    nc.tensor.matmul(
                        pso[:],
                        w1_sb[:, ht, jt * 128 : (jt + 1) * 128],
                        frelu_sb[:, ht, :],
                        start=(ht == 0),
                        stop=(ht == HT - 1),
                    )
                ot = evacp.tile([128, NB], f32, tag="ev")
                nc.scalar.activation(
                    ot[:], pso[:], Identity, bias=b1_sb[:, jt : jt + 1]
                )
                nc.vector.tensor_add(ot[:], ot[:], inpsT_sb[:, jt, :].bitcast(f32))
                nc.sync.dma_start(outT[jt * 128 : (jt + 1) * 128, :], ot[:])

    nc.compile()
    nc.finalize()
    _BUILT["nc"] = nc
    return nc


def _round_fp32r(a):
    """Round fp32 to the fp32r (TF32-like, 1s+8e+11m in top 20 bits) format
    with round-to-nearest-even, as the PE's fp32r datapath expects."""
    b = np.ascontiguousarray(a, np.float32).view(np.uint32).astype(np.uint64)
    lsb = (b >> 12) & 1
    r = ((b + 0x7FF + lsb) & 0xFFFFF000).astype(np.uint32)
    return r.view(np.float32)


def _make_in_maps(inps, fw_adjs, bw_adjs, W_fw, b_fw, W_bw, b_bw, W1, b1):
    f = np.float32
    inps = np.asarray(inps, f)
    W1 = _round_fp32r(np.asarray(W1, f))
    wst = _round_fp32r(
        np.concatenate([np.asarray(W_bw, f), np.asarray(W_fw, f)], axis=0)
    )
    b_cat = np.concatenate([np.asarray(b_bw, f), np.asarray(b_fw, f)], axis=1)  # [R, H]
    bstack = np.ascontiguousarray(b_cat.T.reshape(4, 128, R))
    b1s = np.ascontiguousarray(np.asarray(b1, f).reshape(4, 128, 1))
    fw_adjs = np.asarray(fw_adjs, f)
    bw_adjs = np.asarray(bw_adjs, f)

    in_maps = []
    for c in range(NC):
        sl = slice(c * NB, (c + 1) * NB)
        adjT_c = np.empty((2 * R, NB, N), f)
        for r in range(R):
            adjT_c[r] = bw_adjs[r][:, sl].T
            adjT_c[R + r] = fw_adjs[r][:, sl].T
        in_maps.append(
            {
                "inpsT": _round_fp32r(inps[sl].T),
                "adjT": _round_fp32r(adjT_c),
                "wst": wst,
                "bstack": bstack,
                "w1": W1,
                "b1s": b1s,
            }
        )
    return in_maps


def run(trace=False, **inputs):
    """Run the SPMD kernel; returns (full_output, BassKernelResults)."""
    from concourse.bass_utils import run_bass_kernel_spmd

    nc = _build_nc()
    in_maps = _make_in_maps(**inputs)
    res = run_bass_kernel_spmd(nc, in_maps, core_ids=list(range(NC)), trace=trace)
    out = np.empty((N, H), np.float32)
    for c in range(NC):
        out[c * NB : (c + 1) * NB] = res.results[c]["outT"].T
    return out, res


def kernel(**inputs):
    out, _ = run(trace=False, **inputs)
    return out


# revision 29
# speedup vs baseline: 1.9212x; 1.0866x over previous
"""BiGCN layer kernel for 8 Trainium2 NeuronCores.

Strategy (column-parallel SpMM + ReduceScatter):
  - Each core c owns the contraction slice n in [c*512, (c+1)*512) of all six
    adjacency matrices (3 bw + 3 fw), pre-transposed on host to [n_loc, m] so
    that the contraction dim lands on SBUF partitions with no on-chip
    transposes.
  - sup[r] = inps @ W[r] is computed locally per core for its n-slice only
    (lhsT = inps[block].T, uploaded once and reused for the residual add).
  - feats^T partial = sum_r sup_slice[r].T-contract adjT[r] accumulates all
    relations directly in PSUM; a ReduceScatter over the 8 cores sums the
    partials and hands core c its own m-block.
  - bias+relu fuse into one scalar-engine activation (bias is per-partition
    since feats is produced transposed: [h, m]); final linear contracts h with
    W1 tiles as stationary operands; residual adds inps^T; each core writes
    its [512, 512] transposed output block, assembled on host.
"""

import numpy as np

N, H, R = 4096, 512, 3
K = H // 2            # 256
NC = 8                # cores
NB = N // NC          # 512 rows (m / n_loc) per core
MC = 1024             # m-chunk width streamed per PSUM accumulation group
STAGE_F16 = True      # stage RS partials in fp16 (halves collective payload)
F32 = None            # set lazily to mybir.dt.float32

_BUILT = {}


def _build_nc():
    """Build (and cache) the Bass program. Identical program on all 8 cores."""
    if "nc" in _BUILT:
        return _BUILT["nc"]

    import concourse.bass as bass
    import concourse.mybir as mybir
    from concourse import bacc, tile

    f32 = mybir.dt.float32
    f32r = mybir.dt.float32r
    nc = bacc.Bacc(None, num_devices=NC)

    inpsT = nc.dram_tensor("inpsT", [H, NB], f32r, kind="ExternalInput")
    adjT = nc.dram_tensor("adjT", [2 * R, NB, N], f32r, kind="ExternalInput")
    wst = nc.dram_tensor("wst", [2 * R, H, K], f32r, kind="ExternalInput")
    bstack = nc.dram_tensor("bstack", [4, 128, R], f32, kind="ExternalInput")
    w1 = nc.dram_tensor("w1", [H, H], f32r, kind="ExternalInput")
    b1s = nc.dram_tensor("b1s", [4, 128, 1], f32, kind="ExternalInput")
    outT = nc.dram_tensor("outT", [H, NB], f32, kind="ExternalOutput")

    HT = H // 128     # 4 h-tiles
    NT = NB // 128    # 4 n_loc tiles
    JT = H // 128     # 4 output j tiles
    NMC = N // MC     # 4 m chunks
    Relu = mybir.ActivationFunctionType.Relu
    Identity = mybir.ActivationFunctionType.Identity

    with tile.TileContext(nc) as tc:
        with (
            tc.tile_pool(name="const", bufs=1) as const,
            tc.tile_pool(name="adjp", bufs=4) as adjp,
            tc.tile_pool(name="evacp", bufs=3) as evacp,
            tc.tile_pool(name="psum", bufs=4, space=bass.MemorySpace.PSUM) as psump,
            tc.tile_pool(name="dram", bufs=1, space="DRAM") as dramp,
        ):
            # ---------------- constants into SBUF ----------------
            inpsT_sb = const.tile([128, HT, NB], f32r)      # [p_h, ht, n_loc/m_loc]
            nc.sync.dma_start(inpsT_sb[:], inpsT[:, :].rearrange("(t p) n -> p t n", p=128))
            wst_sb = const.tile([128, 2 * R, HT, K], f32r)  # [p_h, r, ht, k]
            nc.sync.dma_start(wst_sb[:], wst[:, :, :].rearrange("r (t p) k -> p r t k", p=128))
            w1_sb = const.tile([128, HT, H], f32r)          # [p_h, ht, j]
            nc.sync.dma_start(w1_sb[:], w1[:, :].rearrange("(t p) j -> p t j", p=128))
            bst_sb = const.tile([128, JT, R], f32)
            nc.sync.dma_start(bst_sb[:], bstack[:, :, :].rearrange("t p r -> p t r"))
            b1_sb = const.tile([128, JT], f32)
            nc.sync.dma_start(b1_sb[:], b1s[:, :, :].rearrange("t p o -> p (t o)"))

            # summed (over relations) concat bias, per (p, jt)
            bias_sb = const.tile([128, JT], f32)
            for jt in range(JT):
                nc.vector.tensor_add(
                    bias_sb[:, jt : jt + 1], bst_sb[:, jt, 0:1], bst_sb[:, jt, 1:2]
                )
                nc.vector.tensor_add(
                    bias_sb[:, jt : jt + 1], bias_sb[:, jt : jt + 1], bst_sb[:, jt, 2:3]
                )

            # ---------------- local supports: sup[r][n_loc, k] ----------------
            sup_sb = const.tile([128, 2 * R, NT, K], f32r)  # [p_n, r, nt, k]
            for r in range(2 * R):
                for nt in range(NT):
                    ps = psump.tile([128, K], f32, tag="pb")
                    for ht in range(HT):
                        nc.tensor.matmul(
                            ps[:],
                            inpsT_sb[:, ht, nt * 128 : (nt + 1) * 128],
                            wst_sb[:, r, ht, :],
                            start=(ht == 0),
                            stop=(ht == HT - 1),
                        )
                    nc.vector.tensor_copy(sup_sb[:, r, nt, :], ps[:])

            # ---------------- adjacency stream + RS staging ----------------
            # Two separate staging tensors: one shared tile would make the fw
            # stream's staging writes serialize behind RS_bw's read.
            fstag = mybir.dt.float16 if STAGE_F16 else f32
            stag0 = dramp.tile([NC, K, NB], fstag, name="stag0", tag="stag0")
            stag1 = dramp.tile([NC, K, NB], fstag, name="stag1", tag="stag1")
            stags = [stag0, stag1]
            rs_out = []
            for dirn in range(2):                           # 0 = bw (h 0:256), 1 = fw
                stag = stags[dirn]
                for mc in range(NMC):
                    ps0 = psump.tile([128, MC], f32, tag="pb")   # k 0:128
                    ps1 = psump.tile([128, MC], f32, tag="pb")   # k 128:256
                    for ri in range(R):
                        r = dirn * R + ri
                        at = adjp.tile([128, NT, MC], f32r, tag="adj")
                        nc.sync.dma_start(
                            at[:],
                            adjT[r, :, mc * MC : (mc + 1) * MC].rearrange(
                                "(t p) m -> p t m", p=128
                            ),
                        )
                        for nt in range(NT):
                            first = ri == 0 and nt == 0
                            last = ri == R - 1 and nt == NT - 1
                            for kk, ps in ((0, ps0), (1, ps1)):
                                lhsT = sup_sb[:, r, nt, kk * 128 : (kk + 1) * 128]
                                for mh in range(MC // 512):
                                    nc.tensor.matmul(
                                        ps[:, mh * 512 : (mh + 1) * 512],
                                        lhsT,
                                        at[:, nt, mh * 512 : (mh + 1) * 512],
                                        start=first,
                                        stop=last,
                                    )
                    for kk, ps in ((0, ps0), (1, ps1)):
                        ev = evacp.tile([128, MC], fstag, tag="ev")
                        nc.vector.tensor_copy(ev[:], ps[:])
                        for d2 in range(MC // NB):
                            dest = (mc * MC) // NB + d2
                            nc.sync.dma_start(
                                stag[dest, kk * 128 : (kk + 1) * 128, :],
                                ev[:, d2 * NB : (d2 + 1) * NB],
                            )
                ro = dramp.tile(
                    [1, K, NB], fstag, name=f"rs_out{dirn}", tag=f"rs_out{dirn}"
                )
                rs_out.append(ro)
                nc.gpsimd.collective_compute(
                    "ReduceScatter",
                    mybir.AluOpType.add,
                    replica_groups=[list(range(NC))],
                    ins=[stag[:].opt()],
                    outs=[ro[:].opt()],
                )

            # ---------------- bias + relu + final linear + residual ----------------
            # The final matmul accumulates per h-direction so the bw half
            # (frelu ht 0,1 from RS_bw) runs while RS_fw is still in flight.
            frelu_sb = const.tile([128, HT, NB], f32r)      # [p_h, ht, m_loc]
            psos = []
            for half in range(2):                           # 0: ht 0,1 (bw), 1: ht 2,3
                for jt2 in range(2):
                    jt = half * 2 + jt2
                    dirn, row = half, jt2 * 128
                    ft = evacp.tile([128, NB], fstag, tag="ftmp")
                    nc.sync.dma_start(ft[:], rs_out[dirn][0, row : row + 128, :])
                    nc.scalar.activation(
                        frelu_sb[:, jt, :], ft[:], Relu, bias=bias_sb[:, jt : jt + 1]
                    )
                for jt in range(JT):
                    if half == 0:
                        psos.append(
                            psump.tile([128, NB], f32, tag="pb", name=f"pso{jt}")
                        )
                    pso = psos[jt]
                    for ht in (half * 2, half * 2 + 1):
                        nc.tensor.matmul(
                            pso[:],
                            w1_sb[:, ht, jt * 128 : (jt + 1) * 128],
                            frelu_sb[:, ht, :],
                            start=(ht == 0),
                            stop=(ht == HT - 1),
                        )
            for jt in range(JT):
                ot = evacp.tile([128, NB], f32, tag="ev")
                nc.scalar.activation(
                    ot[:], psos[jt][:], Identity, bias=b1_sb[:, jt : jt + 1]
                )
                nc.vector.tensor_add(ot[:], ot[:], inpsT_sb[:, jt, :].bitcast(f32))
                nc.sync.dma_start(outT[jt * 128 : (jt + 1) * 128, :], ot[:])

    nc.compile()
    nc.finalize()
    _BUILT["nc"] = nc
    return nc


def _round_fp32r(a):
    """Round fp32 to the fp32r (TF32-like, 1s+8e+11m in top 20 bits) format
    with round-to-nearest-even, as the PE's fp32r datapath expects."""
    b = np.ascontiguousarray(a, np.float32).view(np.uint32).astype(np.uint64)
    lsb = (b >> 12) & 1
    r = ((b + 0x7FF + lsb) & 0xFFFFF000).astype(np.uint32)
    return r.view(np.float32)


def _make_in_maps(inps, fw_adjs, bw_adjs, W_fw, b_fw, W_bw, b_bw, W1, b1):
    f = np.float32
    inps = np.asarray(inps, f)
    W1 = _round_fp32r(np.asarray(W1, f))
    wst = _round_fp32r(
        np.concatenate([np.asarray(W_bw, f), np.asarray(W_fw, f)], axis=0)
    )
    b_cat = np.concatenate([np.asarray(b_bw, f), np.asarray(b_fw, f)], axis=1)  # [R, H]
    bstack = np.ascontiguousarray(b_cat.T.reshape(4, 128, R))
    b1s = np.ascontiguousarray(np.asarray(b1, f).reshape(4, 128, 1))
    fw_adjs = np.asarray(fw_adjs, f)
    bw_adjs = np.asarray(bw_adjs, f)

    in_maps = []
    for c in range(NC):
        sl = slice(c * NB, (c + 1) * NB)
        adjT_c = np.empty((2 * R, NB, N), f)
        for r in range(R):
            adjT_c[r] = bw_adjs[r][:, sl].T
            adjT_c[R + r] = fw_adjs[r][:, sl].T
        in_maps.append(
            {
                "inpsT": _round_fp32r(inps[sl].T),
                "adjT": _round_fp32r(adjT_c),
                "wst": wst,
                "bstack": bstack,
                "w1": W1,
                "b1s": b1s,
            }
        )
    return in_maps


def run(trace=False, **inputs):
    """Run the SPMD kernel; returns (full_output, BassKernelResults)."""
    from concourse.bass_utils import run_bass_kernel_spmd

    nc = _build_nc()
    in_maps = _make_in_maps(**inputs)
    res = run_bass_kernel_spmd(nc, in_maps, core_ids=list(range(NC)), trace=trace)
    out = np.empty((N, H), np.float32)
    for c in range(NC):
        out[c * NB : (c + 1) * NB] = res.results[c]["outT"].T
    return out, res


def kernel(**inputs):
    out, _ = run(trace=False, **inputs)
    return out


# revision 37
# speedup vs baseline: 2.7243x; 1.4180x over previous
"""BiGCN layer kernel for 8 Trainium2 NeuronCores.

Strategy (column-parallel SpMM + ReduceScatter):
  - Each core c owns the contraction slice n in [c*512, (c+1)*512) of all six
    adjacency matrices (3 bw + 3 fw), pre-transposed on host to [n_loc, m] so
    that the contraction dim lands on SBUF partitions with no on-chip
    transposes.
  - sup[r] = inps @ W[r] is computed locally per core for its n-slice only
    (lhsT = inps[block].T, uploaded once and reused for the residual add).
  - feats^T partial = sum_r sup_slice[r].T-contract adjT[r] accumulates all
    relations directly in PSUM; a ReduceScatter over the 8 cores sums the
    partials and hands core c its own m-block.
  - bias+relu fuse into one scalar-engine activation (bias is per-partition
    since feats is produced transposed: [h, m]); final linear contracts h with
    W1 tiles as stationary operands; residual adds inps^T; each core writes
    its [512, 512] transposed output block, assembled on host.
"""

import numpy as np

N, H, R = 4096, 512, 3
K = H // 2            # 256
NC = 8                # cores
NB = N // NC          # 512 rows (m / n_loc) per core
MC = 1024             # m-chunk width streamed per PSUM accumulation group
STAGE_F16 = True      # stage RS partials in fp16 (halves collective payload)
ADJ_F16 = True        # stream adjacencies (and sup) as fp16: halves HBM traffic;
                      # fp16's 11-bit mantissa matches fp32r's precision class
                      # for these [0,1) adjacency values
F32 = None            # set lazily to mybir.dt.float32

_BUILT = {}


def _build_nc():
    """Build (and cache) the Bass program. Identical program on all 8 cores."""
    if "nc" in _BUILT:
        return _BUILT["nc"]

    import concourse.bass as bass
    import concourse.mybir as mybir
    from concourse import bacc, tile

    f32 = mybir.dt.float32
    f32r = mybir.dt.float32r
    f16 = mybir.dt.float16
    fadj = f16 if ADJ_F16 else f32r
    nc = bacc.Bacc(None, num_devices=NC)

    inpsT = nc.dram_tensor("inpsT", [H, NB], fadj, kind="ExternalInput")
    inpsR = nc.dram_tensor("inpsR", [H, NB], f32, kind="ExternalInput")
    adjT = nc.dram_tensor("adjT", [2 * R, NB, N], fadj, kind="ExternalInput")
    wst = nc.dram_tensor("wst", [2 * R, H, K], fadj, kind="ExternalInput")
    bstack = nc.dram_tensor("bstack", [4, 128, R], f32, kind="ExternalInput")
    w1 = nc.dram_tensor("w1", [H, H], f32r, kind="ExternalInput")
    b1s = nc.dram_tensor("b1s", [4, 128, 1], f32, kind="ExternalInput")
    outT = nc.dram_tensor("outT", [H, NB], f32, kind="ExternalOutput")

    HT = H // 128     # 4 h-tiles
    NT = NB // 128    # 4 n_loc tiles
    JT = H // 128     # 4 output j tiles
    NMC = N // MC     # 4 m chunks
    Relu = mybir.ActivationFunctionType.Relu
    Identity = mybir.ActivationFunctionType.Identity

    with tile.TileContext(nc) as tc:
        with (
            tc.tile_pool(name="const", bufs=1) as const,
            tc.tile_pool(name="adjp", bufs=4) as adjp,
            tc.tile_pool(name="evacp", bufs=3) as evacp,
            tc.tile_pool(name="psum", bufs=4, space=bass.MemorySpace.PSUM) as psump,
            tc.tile_pool(name="dram", bufs=1, space="DRAM") as dramp,
        ):
            # ---------------- constants into SBUF ----------------
            inpsT_sb = const.tile([128, HT, NB], fadj)      # [p_h, ht, n_loc/m_loc]
            nc.sync.dma_start(inpsT_sb[:], inpsT[:, :].rearrange("(t p) n -> p t n", p=128))
            inpsR_sb = const.tile([128, HT, NB], f32)       # exact fp32 for residual
            nc.sync.dma_start(inpsR_sb[:], inpsR[:, :].rearrange("(t p) n -> p t n", p=128))
            wst_sb = const.tile([128, 2 * R, HT, K], fadj)  # [p_h, r, ht, k]
            nc.sync.dma_start(wst_sb[:], wst[:, :, :].rearrange("r (t p) k -> p r t k", p=128))
            w1_sb = const.tile([128, HT, H], f32r)          # [p_h, ht, j]
            nc.sync.dma_start(w1_sb[:], w1[:, :].rearrange("(t p) j -> p t j", p=128))
            bst_sb = const.tile([128, JT, R], f32)
            nc.sync.dma_start(bst_sb[:], bstack[:, :, :].rearrange("t p r -> p t r"))
            b1_sb = const.tile([128, JT], f32)
            nc.sync.dma_start(b1_sb[:], b1s[:, :, :].rearrange("t p o -> p (t o)"))

            # summed (over relations) concat bias, per (p, jt)
            bias_sb = const.tile([128, JT], f32)
            for jt in range(JT):
                nc.vector.tensor_add(
                    bias_sb[:, jt : jt + 1], bst_sb[:, jt, 0:1], bst_sb[:, jt, 1:2]
                )
                nc.vector.tensor_add(
                    bias_sb[:, jt : jt + 1], bias_sb[:, jt : jt + 1], bst_sb[:, jt, 2:3]
                )

            # ---------------- local supports: sup[r][n_loc, k] ----------------
            sup_sb = const.tile([128, 2 * R, NT, K], fadj)  # [p_n, r, nt, k]
            for r in range(2 * R):
                for nt in range(NT):
                    ps = psump.tile([128, K], f32, tag="pb")
                    for ht in range(HT):
                        nc.tensor.matmul(
                            ps[:],
                            inpsT_sb[:, ht, nt * 128 : (nt + 1) * 128],
                            wst_sb[:, r, ht, :],
                            start=(ht == 0),
                            stop=(ht == HT - 1),
                        )
                    nc.vector.tensor_copy(sup_sb[:, r, nt, :], ps[:])

            # ---------------- adjacency stream + RS staging ----------------
            # Two separate staging tensors: one shared tile would make the fw
            # stream's staging writes serialize behind RS_bw's read.
            fstag = mybir.dt.float16 if STAGE_F16 else f32
            stag0 = dramp.tile([NC, K, NB], fstag, name="stag0", tag="stag0")
            stag1 = dramp.tile([NC, K, NB], fstag, name="stag1", tag="stag1")
            stags = [stag0, stag1]
            rs_out = []
            for dirn in range(2):                           # 0 = bw (h 0:256), 1 = fw
                stag = stags[dirn]
                for mc in range(NMC):
                    ps0 = psump.tile([128, MC], f32, tag="pb")   # k 0:128
                    ps1 = psump.tile([128, MC], f32, tag="pb")   # k 128:256
                    for ri in range(R):
                        r = dirn * R + ri
                        at = adjp.tile([128, NT, MC], fadj, tag="adj")
                        nc.sync.dma_start(
                            at[:],
                            adjT[r, :, mc * MC : (mc + 1) * MC].rearrange(
                                "(t p) m -> p t m", p=128
                            ),
                        )
                        for nt in range(NT):
                            first = ri == 0 and nt == 0
                            last = ri == R - 1 and nt == NT - 1
                            for kk, ps in ((0, ps0), (1, ps1)):
                                lhsT = sup_sb[:, r, nt, kk * 128 : (kk + 1) * 128]
                                for mh in range(MC // 512):
                                    nc.tensor.matmul(
                                        ps[:, mh * 512 : (mh + 1) * 512],
                                        lhsT,
                                        at[:, nt, mh * 512 : (mh + 1) * 512],
                                        start=first,
                                        stop=last,
                                    )
                    for kk, ps in ((0, ps0), (1, ps1)):
                        ev = evacp.tile([128, MC], fstag, tag="ev")
                        nc.vector.tensor_copy(ev[:], ps[:])
                        for d2 in range(MC // NB):
                            dest = (mc * MC) // NB + d2
                            nc.sync.dma_start(
                                stag[dest, kk * 128 : (kk + 1) * 128, :],
                                ev[:, d2 * NB : (d2 + 1) * NB],
                            )
                ro = dramp.tile(
                    [1, K, NB], fstag, name=f"rs_out{dirn}", tag=f"rs_out{dirn}"
                )
                rs_out.append(ro)
                nc.gpsimd.collective_compute(
                    "ReduceScatter",
                    mybir.AluOpType.add,
                    replica_groups=[list(range(NC))],
                    ins=[stag[:].opt()],
                    outs=[ro[:].opt()],
                )

            # ---------------- bias + relu + final linear + residual ----------------
            # The final matmul accumulates per h-direction so the bw half
            # (frelu ht 0,1 from RS_bw) runs while RS_fw is still in flight.
            frelu_sb = const.tile([128, HT, NB], f32r)      # [p_h, ht, m_loc]
            psos = []
            for half in range(2):                           # 0: ht 0,1 (bw), 1: ht 2,3
                for jt2 in range(2):
                    jt = half * 2 + jt2
                    dirn, row = half, jt2 * 128
                    ft = evacp.tile([128, NB], fstag, tag="ftmp")
                    nc.sync.dma_start(ft[:], rs_out[dirn][0, row : row + 128, :])
                    nc.scalar.activation(
                        frelu_sb[:, jt, :], ft[:], Relu, bias=bias_sb[:, jt : jt + 1]
                    )
                for jt in range(JT):
                    if half == 0:
                        psos.append(
                            psump.tile([128, NB], f32, tag="pb", name=f"pso{jt}")
                        )
                    pso = psos[jt]
                    for ht in (half * 2, half * 2 + 1):
                        nc.tensor.matmul(
                            pso[:],
                            w1_sb[:, ht, jt * 128 : (jt + 1) * 128],
                            frelu_sb[:, ht, :],
                            start=(ht == 0),
                            stop=(ht == HT - 1),
                        )
            for jt in range(JT):
                ot = evacp.tile([128, NB], f32, tag="ev")
                nc.scalar.activation(
                    ot[:], psos[jt][:], Identity, bias=b1_sb[:, jt : jt + 1]
                )
                nc.vector.tensor_add(ot[:], ot[:], inpsR_sb[:, jt, :])
                nc.sync.dma_start(outT[jt * 128 : (jt + 1) * 128, :], ot[:])

    nc.compile()
    nc.finalize()
    _BUILT["nc"] = nc
    return nc


def _round_fp32r(a):
    """Round fp32 to the fp32r (TF32-like, 1s+8e+11m in top 20 bits) format
    with round-to-nearest-even, as the PE's fp32r datapath expects."""
    b = np.ascontiguousarray(a, np.float32).view(np.uint32).astype(np.uint64)
    lsb = (b >> 12) & 1
    r = ((b + 0x7FF + lsb) & 0xFFFFF000).astype(np.uint32)
    return r.view(np.float32)


def _make_in_maps(inps, fw_adjs, bw_adjs, W_fw, b_fw, W_bw, b_bw, W1, b1):
    f = np.float32
    fadj = np.float16 if ADJ_F16 else f
    _round_adj = (lambda a: np.ascontiguousarray(a, np.float16)) if ADJ_F16 else _round_fp32r
    inps = np.asarray(inps, f)
    W1 = _round_fp32r(np.asarray(W1, f))
    wst = _round_adj(
        np.concatenate([np.asarray(W_bw, f), np.asarray(W_fw, f)], axis=0)
    )
    b_cat = np.concatenate([np.asarray(b_bw, f), np.asarray(b_fw, f)], axis=1)  # [R, H]
    bstack = np.ascontiguousarray(b_cat.T.reshape(4, 128, R))
    b1s = np.ascontiguousarray(np.asarray(b1, f).reshape(4, 128, 1))
    fw_adjs = np.asarray(fw_adjs, f)
    bw_adjs = np.asarray(bw_adjs, f)

    in_maps = []
    for c in range(NC):
        sl = slice(c * NB, (c + 1) * NB)
        adjT_c = np.empty((2 * R, NB, N), fadj)
        for r in range(R):
            adjT_c[r] = bw_adjs[r][:, sl].T
            adjT_c[R + r] = fw_adjs[r][:, sl].T
        in_maps.append(
            {
                "inpsT": _round_adj(inps[sl].T),
                "inpsR": np.ascontiguousarray(inps[sl].T),
                "adjT": adjT_c,
                "wst": wst,
                "bstack": bstack,
                "w1": W1,
                "b1s": b1s,
            }
        )
    return in_maps


def run(trace=False, **inputs):
    """Run the SPMD kernel; returns (full_output, BassKernelResults)."""
    from concourse.bass_utils import run_bass_kernel_spmd

    nc = _build_nc()
    in_maps = _make_in_maps(**inputs)
    res = run_bass_kernel_spmd(nc, in_maps, core_ids=list(range(NC)), trace=trace)
    out = np.empty((N, H), np.float32)
    for c in range(NC):
        out[c * NB : (c + 1) * NB] = res.results[c]["outT"].T
    return out, res


def kernel(**inputs):
    out, _ = run(trace=False, **inputs)
    return out
